# revision 4
# baseline (speedup 1.0000x reference)
"""Trainium2 Bass kernel for nn_Jitter: out[:, i, :] = x[:, indices[i], :].

Full shapes: x (64, 4096, 256) f32, indices (4096,) int64 -> out (64, 4096, 256) f32.

Strategy: data-parallel over batch dim across 8 NeuronCores (8 batches per
core); the tiny index vector is replicated to every core. On each core the
time-axis gather uses the SWDGE `dma_gather` ucode instruction (production
embedding-gather path): one instruction gathers all 4096 rows (1KB each) of
one batch into a [128, 32, 256] SBUF tile (index n -> partition n%128, chunk
n//128), which an HWDGE DMA then stores to the matching interleaved view of
the output. Memory-bound: each core moves 32MB in + 32MB out.

Indices for dma_gather are int16, wrapped into 16 partitions (idx n ->
partition n%16, col n//16) and replicated to all 128 partitions for the 8
GpSimd cores.
"""

import numpy as np

import concourse.bass as bass
import concourse.tile as tile
from concourse import bacc, mybir
from concourse.bass_utils import run_bass_kernel_spmd
from concourse.library_config import mlp as _mlp_lib

N_CORES = 8
B, T, C = 64, 4096, 256
B_LOC = B // N_CORES  # 8 batches per core
P = 128               # SBUF partitions
J = T // P            # 32 gathered rows per partition
JW = T // 16          # idx tile cols (16-partition wrap)

_CACHE = {}

# The SWDGE descriptor ring holds 1024 descriptors (dynamic_dma_scratch_size
# 16384 / 16B); one dma_gather must stay under that, so split each batch's
# 4096 indices into 4 sub-gathers of 1024.
GSPLIT = 4
IDX_PER_G = T // GSPLIT          # 1024 indices per gather instruction
JW_PER_G = JW // GSPLIT          # 64 idx-tile cols per gather
J_PER_G = J // GSPLIT            # 8 output chunks per gather


def _build(repeat: int = 1, bufs: int = 4):
    """Build + compile the per-core SPMD program.

    repeat: run the whole gather body `repeat` times (for wall-clock delta
            timing in test harnesses); the result is unchanged.
    """
    nc = bacc.Bacc("TRN2", target_bir_lowering=False, debug=False,
                   num_devices=N_CORES)
    x_ext = nc.dram_tensor("x", [B_LOC, T, C], mybir.dt.float32,
                           kind="ExternalInput").ap()
    idx_ext = nc.dram_tensor("idx", [P, JW], mybir.dt.int16,
                             kind="ExternalInput").ap()
    out_ext = nc.dram_tensor("out", [B_LOC, T, C], mybir.dt.float32,
                             kind="ExternalOutput").ap()

    with tile.TileContext(nc) as tc:
        with tc.tile_pool(name="idxp", bufs=1) as idx_pool, \
             tc.tile_pool(name="data", bufs=bufs) as data_pool:
            nc.gpsimd.load_library(_mlp_lib)
            idx_t = idx_pool.tile([P, JW], mybir.dt.int16)
            nc.sync.dma_start(out=idx_t[:], in_=idx_ext[:])
            for _ in range(repeat):
                for b in range(B_LOC):
                    dt = data_pool.tile([P, J, C], mybir.dt.float32)
                    for g in range(GSPLIT):
                        # slots n in [g*1024, (g+1)*1024): local m = n - g*1024
                        # lands at [m % 128, m // 128] of the slice, which is
                        # [n % 128, n // 128] of the full tile (1024 % 128 == 0).
                        # The host permutes the idx vector so slot n carries
                        # output row (n%128)*32 + n//128 -> dt[p, j] = row
                        # p*32 + j (block layout), making the store contiguous.
                        nc.gpsimd.dma_gather(
                            dt[:, g * J_PER_G:(g + 1) * J_PER_G, :],
                            x_ext[b],
                            idx_t[:, g * JW_PER_G:(g + 1) * JW_PER_G],
                            num_idxs=IDX_PER_G, num_idxs_reg=IDX_PER_G,
                            elem_size=C,
                        )
                    # dt[p, j, :] = out row p*32 + j: contiguous store
                    out_view = out_ext[b].rearrange("(p j) c -> p j c", p=P)
                    nc.sync.dma_start(out=out_view, in_=dt[:])
    nc.compile()
    return nc


def _prep_idx(indices: np.ndarray) -> np.ndarray:
    idx16 = indices.astype(np.int16)                    # values < 4096 fit
    # gather slot n lands at dt[n % 128, n // 128]; permute so slot n
    # carries output row (n % 128) * 32 + (n // 128) -> block layout.
    n = np.arange(T)
    idx_perm = idx16[(n % P) * J + (n // P)]
    wrapped = np.ascontiguousarray(idx_perm.reshape(JW, 16).T)   # [16, JW]
    return np.ascontiguousarray(np.tile(wrapped, (P // 16, 1)))  # [128, JW]


def _make_in_maps(x: np.ndarray, indices: np.ndarray):
    idx_arr = _prep_idx(np.asarray(indices))
    x = np.asarray(x)
    return [
        {"x": np.ascontiguousarray(x[i * B_LOC:(i + 1) * B_LOC]),
         "idx": idx_arr}
        for i in range(N_CORES)
    ]


def _gather_out(res) -> np.ndarray:
    return np.concatenate([res.results[i]["out"] for i in range(N_CORES)],
                          axis=0)


def kernel(x: np.ndarray, indices: np.ndarray) -> np.ndarray:
    key = "main"
    if key not in _CACHE:
        _CACHE[key] = _build()
    nc = _CACHE[key]

    in_maps = _make_in_maps(x, indices)
    res = run_bass_kernel_spmd(nc, in_maps, list(range(N_CORES)))
    return _gather_out(res)



# revision 13
# speedup vs baseline: 1.3546x; 1.3546x over previous
"""Trainium2 Bass kernel for nn_Jitter: out[:, i, :] = x[:, indices[i], :].

Full shapes: x (64, 4096, 256) f32, indices (4096,) int64 -> out (64, 4096, 256) f32.

Sharding: data-parallel over batch dim across 8 NeuronCores (8 batches per
core); the tiny index vector is replicated to every core. Memory-bound:
each core moves ~32MB in + 32MB out.

Primary variant "shift": jitter indices satisfy idx[i] in {i-1, i, i+1}
(boundary-forced), so the gather is out = select(m_minus, x[-1], m_plus,
x[+1], else x[0]) — three row-shifted views of the same loaded tile.
Each SBUF partition p loads a 34-row window [32p-1 .. 32p+32] of x (the
host pads x by one leading dummy row so the window AP is in-bounds), the
ACT engine copies the center view, the DVE predicated-overwrites the
+-1 rows using host-precomputed uint8 masks, and the result stores back
contiguously. All HBM traffic is sequential (no gather descriptors), and
loads/stores share one HWDGE ring in software-pipelined order so reads
and writes hit HBM in long same-direction bursts.

Fallback variants use the SWDGE `dma_gather` ucode (1KB row descriptors)
and handle arbitrary indices; "ident"/"block"/"ident_jstore" differ only
in gather-slot/store layout. Used if indices are not jitter-structured.
"""

import numpy as np

import concourse.bass as bass
import concourse.tile as tile
from concourse.ap import AP
from concourse import bacc, mybir
from concourse.bass_utils import run_bass_kernel_spmd
from concourse.library_config import mlp as _mlp_lib

N_CORES = 8
B, T, C = 64, 4096, 256
B_LOC = B // N_CORES  # 8 batches per core
P = 128               # SBUF partitions
J = T // P            # 32 rows per partition
JW = T // 16          # idx tile cols (16-partition wrap)
WIN = J + 2           # per-partition load window (1 halo row each side)
ROWS_PAD = B_LOC * T + 64  # 1 leading dummy row + data + tail pad

_CACHE = {}

VARIANT = "shift"

# SWDGE descriptor ring holds 1024 descriptors; one dma_gather must stay
# under that, so the gather fallback splits 4096 indices into 4 sub-gathers.
GSPLIT = 4
IDX_PER_G = T // GSPLIT
JW_PER_G = JW // GSPLIT
J_PER_G = J // GSPLIT


def _build_shift(repeat: int = 1, bufs: int = 2):
    nc = bacc.Bacc("TRN2", target_bir_lowering=False, debug=False,
                   num_devices=N_CORES)
    xp_ext = nc.dram_tensor("xp", [ROWS_PAD, C], mybir.dt.float32,
                            kind="ExternalInput").ap()
    mm_ext = nc.dram_tensor("mm", [P, J, C], mybir.dt.uint8,
                            kind="ExternalInput").ap()
    mp_ext = nc.dram_tensor("mp", [P, J, C], mybir.dt.uint8,
                            kind="ExternalInput").ap()
    out_ext = nc.dram_tensor("out", [B_LOC, T, C], mybir.dt.float32,
                             kind="ExternalOutput").ap()

    with tile.TileContext(nc) as tc:
        with tc.tile_pool(name="maskp", bufs=1) as mask_pool, \
             tc.tile_pool(name="xin", bufs=3) as x_pool, \
             tc.tile_pool(name="oout", bufs=2) as o_pool:
            mm_t = mask_pool.tile([P, J, C], mybir.dt.uint8)
            mp_t = mask_pool.tile([P, J, C], mybir.dt.uint8)
            nc.sync.dma_start(out=mm_t[:], in_=mm_ext[:])
            nc.sync.dma_start(out=mp_t[:], in_=mp_ext[:])
            # All data DMA rides one HWDGE ring (SP), software-pipelined so
            # the ring drains L(0),L(1),S(0),L(2),S(1),...: long same-direction
            # HBM bursts (loads vs stores) instead of packet-level read/write
            # interleaving, worth ~25us/iter; compute overlaps on ACT/DVE.
            pend = None  # pending (batch, ot_ap) store
            for _ in range(repeat):
                for b in range(B_LOC):
                    xt = x_pool.tile([P, WIN, C], mybir.dt.float32)
                    # partition p <- padded rows [b*T + 32p, b*T + 32p + 34)
                    # = original rows [b*T + 32p - 1, b*T + 32p + 33)
                    src = AP(xp_ext.tensor, b * T * C,
                             [(J * C, P), (C, WIN), (1, C)])
                    nc.sync.dma_start(out=xt[:], in_=src)
                    if pend is not None:
                        ov = out_ext[pend[0]].rearrange("(p j) c -> p j c", p=P)
                        nc.sync.dma_start(out=ov, in_=pend[1])
                    ot = o_pool.tile([P, J, C], mybir.dt.float32)
                    nc.scalar.copy(ot[:], xt[:, 1:J + 1, :])
                    nc.vector.copy_predicated(ot[:], mm_t[:], xt[:, 0:J, :])
                    nc.vector.copy_predicated(ot[:], mp_t[:], xt[:, 2:J + 2, :])
                    pend = (b, ot[:])
            if pend is not None:
                ov = out_ext[pend[0]].rearrange("(p j) c -> p j c", p=P)
                nc.sync.dma_start(out=ov, in_=pend[1])
    nc.compile()
    return nc


def _build_gather(repeat: int = 1, bufs: int = 4, variant: str = "ident"):
    nc = bacc.Bacc("TRN2", target_bir_lowering=False, debug=False,
                   num_devices=N_CORES)
    x_ext = nc.dram_tensor("x", [B_LOC, T, C], mybir.dt.float32,
                           kind="ExternalInput").ap()
    idx_ext = nc.dram_tensor("idx", [P, JW], mybir.dt.int16,
                             kind="ExternalInput").ap()
    out_ext = nc.dram_tensor("out", [B_LOC, T, C], mybir.dt.float32,
                             kind="ExternalOutput").ap()

    with tile.TileContext(nc) as tc:
        with tc.tile_pool(name="idxp", bufs=1) as idx_pool, \
             tc.tile_pool(name="data", bufs=bufs) as data_pool:
            nc.gpsimd.load_library(_mlp_lib)
            idx_t = idx_pool.tile([P, JW], mybir.dt.int16)
            nc.sync.dma_start(out=idx_t[:], in_=idx_ext[:])
            for _ in range(repeat):
                for b in range(B_LOC):
                    dt = data_pool.tile([P, J, C], mybir.dt.float32)
                    for g in range(GSPLIT):
                        # gather slot n -> dt[n % 128, n // 128]
                        nc.gpsimd.dma_gather(
                            dt[:, g * J_PER_G:(g + 1) * J_PER_G, :],
                            x_ext[b],
                            idx_t[:, g * JW_PER_G:(g + 1) * JW_PER_G],
                            num_idxs=IDX_PER_G, num_idxs_reg=IDX_PER_G,
                            elem_size=C,
                        )
                    if variant == "block":
                        # host permuted slots so dt[p, j] = out row p*32 + j
                        out_view = out_ext[b].rearrange("(p j) c -> p j c", p=P)
                        nc.sync.dma_start(out=out_view, in_=dt[:])
                    elif variant == "ident":
                        # dt[p, j] = out row j*128 + p: interleaved store
                        out_view = out_ext[b].rearrange("(j p) c -> p j c", p=P)
                        nc.sync.dma_start(out=out_view, in_=dt[:])
                    elif variant == "ident_jstore":
                        ov = out_ext[b].rearrange("(j p) c -> j p c", p=P)
                        for j in range(J):
                            nc.sync.dma_start(out=ov[j], in_=dt[:, j, :])
                    else:
                        raise ValueError(variant)
    nc.compile()
    return nc


def _build(repeat: int = 1, bufs: int | None = None, variant: str | None = None):
    variant = variant or VARIANT
    if variant == "shift":
        return _build_shift(repeat=repeat, bufs=bufs or 2)
    return _build_gather(repeat=repeat, bufs=bufs or 4, variant=variant)


def _prep_idx(indices: np.ndarray, variant: str = "ident") -> np.ndarray:
    idx16 = indices.astype(np.int16)                    # values < 4096 fit
    if variant == "block":
        n = np.arange(T)
        idx16 = idx16[(n % P) * J + (n // P)]
    wrapped = np.ascontiguousarray(idx16.reshape(JW, 16).T)   # [16, JW]
    return np.ascontiguousarray(np.tile(wrapped, (P // 16, 1)))  # [128, JW]


def _prep_masks(indices: np.ndarray):
    d = indices.astype(np.int64) - np.arange(T)
    mm = (d == -1).astype(np.uint8)   # take row t-1
    mp = (d == 1).astype(np.uint8)    # take row t+1
    def expand(m):
        full = np.repeat(m.reshape(P, J), C, axis=1).reshape(P, J, C)
        return np.ascontiguousarray(full)
    return expand(mm), expand(mp)


def _make_in_maps(x: np.ndarray, indices: np.ndarray, variant: str | None = None):
    variant = variant or VARIANT
    x = np.asarray(x)
    indices = np.asarray(indices)
    if variant == "shift":
        mm, mp = _prep_masks(indices)
        maps = []
        for i in range(N_CORES):
            flat = x[i * B_LOC:(i + 1) * B_LOC].reshape(B_LOC * T, C)
            xp = np.zeros((ROWS_PAD, C), dtype=np.float32)
            xp[1:1 + B_LOC * T] = flat
            maps.append({"xp": xp, "mm": mm, "mp": mp})
        return maps
    idx_arr = _prep_idx(indices, variant)
    return [
        {"x": np.ascontiguousarray(x[i * B_LOC:(i + 1) * B_LOC]),
         "idx": idx_arr}
        for i in range(N_CORES)
    ]


def _gather_out(res) -> np.ndarray:
    return np.concatenate([res.results[i]["out"] for i in range(N_CORES)],
                          axis=0)


def _is_jitter(indices: np.ndarray) -> bool:
    idx = np.asarray(indices).astype(np.int64)
    if idx.shape != (T,) or idx.min() < 0 or idx.max() >= T:
        return False
    return bool(np.all(np.abs(idx - np.arange(T)) <= 1))


def kernel(x: np.ndarray, indices: np.ndarray) -> np.ndarray:
    variant = VARIANT if _is_jitter(indices) else "ident"
    if variant not in _CACHE:
        _CACHE[variant] = _build(variant=variant)
    nc = _CACHE[variant]

    in_maps = _make_in_maps(x, indices, variant)
    res = run_bass_kernel_spmd(nc, in_maps, list(range(N_CORES)))
    return _gather_out(res)


# revision 15
# speedup vs baseline: 1.4280x; 1.0541x over previous
"""Trainium2 Bass kernel for nn_Jitter: out[:, i, :] = x[:, indices[i], :].

Full shapes: x (64, 4096, 256) f32, indices (4096,) int64 -> out (64, 4096, 256) f32.

Sharding: data-parallel over batch dim across 8 NeuronCores (8 batches per
core); the tiny index vector is replicated to every core. Memory-bound:
each core moves ~32MB in + 32MB out.

Primary variant "shift": jitter indices satisfy idx[i] in {i-1, i, i+1}
(boundary-forced), so the gather is out = select(m_minus, x[-1], m_plus,
x[+1], else x[0]) — three row-shifted views of the same loaded tile.
Each SBUF partition p loads a 34-row window [32p-1 .. 32p+32] of x (the
host pads x by one leading dummy row so the window AP is in-bounds), the
ACT engine copies the center view, the DVE predicated-overwrites the
+-1 rows using host-precomputed uint8 masks, and the result stores back
contiguously. All HBM traffic is sequential (no gather descriptors), and
loads/stores share one SWDGE (gpsimd) queue in software-pipelined order
so reads and writes hit HBM in long same-direction bursts.

Fallback variants use the SWDGE `dma_gather` ucode (1KB row descriptors)
and handle arbitrary indices; "ident"/"block"/"ident_jstore" differ only
in gather-slot/store layout. Used if indices are not jitter-structured.
"""

import numpy as np

import concourse.bass as bass
import concourse.tile as tile
from concourse.ap import AP
from concourse import bacc, mybir
from concourse.bass_utils import run_bass_kernel_spmd
from concourse.library_config import mlp as _mlp_lib

N_CORES = 8
B, T, C = 64, 4096, 256
B_LOC = B // N_CORES  # 8 batches per core
P = 128               # SBUF partitions
J = T // P            # 32 rows per partition
JW = T // 16          # idx tile cols (16-partition wrap)
WIN = J + 2           # per-partition load window (1 halo row each side)
ROWS_PAD = B_LOC * T + 64  # 1 leading dummy row + data + tail pad

_CACHE = {}

VARIANT = "shift"

# SWDGE descriptor ring holds 1024 descriptors; one dma_gather must stay
# under that, so the gather fallback splits 4096 indices into 4 sub-gathers.
GSPLIT = 4
IDX_PER_G = T // GSPLIT
JW_PER_G = JW // GSPLIT
J_PER_G = J // GSPLIT


def _build_shift(repeat: int = 1, bufs: int = 2):
    nc = bacc.Bacc("TRN2", target_bir_lowering=False, debug=False,
                   num_devices=N_CORES)
    xp_ext = nc.dram_tensor("xp", [ROWS_PAD, C], mybir.dt.float32,
                            kind="ExternalInput").ap()
    mm_ext = nc.dram_tensor("mm", [P, J, C], mybir.dt.uint8,
                            kind="ExternalInput").ap()
    mp_ext = nc.dram_tensor("mp", [P, J, C], mybir.dt.uint8,
                            kind="ExternalInput").ap()
    out_ext = nc.dram_tensor("out", [B_LOC, T, C], mybir.dt.float32,
                             kind="ExternalOutput").ap()

    with tile.TileContext(nc) as tc:
        with tc.tile_pool(name="maskp", bufs=1) as mask_pool, \
             tc.tile_pool(name="xin", bufs=3) as x_pool, \
             tc.tile_pool(name="oout", bufs=2) as o_pool:
            nc.gpsimd.load_library(_mlp_lib)
            mm_t = mask_pool.tile([P, J, C], mybir.dt.uint8)
            mp_t = mask_pool.tile([P, J, C], mybir.dt.uint8)
            nc.sync.dma_start(out=mm_t[:], in_=mm_ext[:])
            nc.sync.dma_start(out=mp_t[:], in_=mp_ext[:])
            # All data DMA rides one SWDGE queue (gpsimd), software-pipelined so
            # the ring drains L(0),L(1),S(0),L(2),S(1),...: long same-direction
            # HBM bursts (loads vs stores) instead of packet-level read/write
            # interleaving, worth ~25us/iter; compute overlaps on ACT/DVE.
            pend = None  # pending (batch, ot_ap) store
            for _ in range(repeat):
                for b in range(B_LOC):
                    xt = x_pool.tile([P, WIN, C], mybir.dt.float32)
                    # partition p <- padded rows [b*T + 32p, b*T + 32p + 34)
                    # = original rows [b*T + 32p - 1, b*T + 32p + 33);
                    # two 17-row chunks beat one 34KB descriptor per partition
                    for st_, ln in ((0, 17), (17, 17)):
                        src = AP(xp_ext.tensor, b * T * C + st_ * C,
                                 [(J * C, P), (C, ln), (1, C)])
                        nc.gpsimd.dma_start(out=xt[:, st_:st_ + ln, :], in_=src)
                    if pend is not None:
                        ov = out_ext[pend[0]].rearrange("(p j) c -> p j c", p=P)
                        nc.gpsimd.dma_start(out=ov, in_=pend[1])
                    ot = o_pool.tile([P, J, C], mybir.dt.float32)
                    nc.scalar.copy(ot[:], xt[:, 1:J + 1, :])
                    nc.vector.copy_predicated(ot[:], mm_t[:], xt[:, 0:J, :])
                    nc.vector.copy_predicated(ot[:], mp_t[:], xt[:, 2:J + 2, :])
                    pend = (b, ot[:])
            if pend is not None:
                ov = out_ext[pend[0]].rearrange("(p j) c -> p j c", p=P)
                nc.gpsimd.dma_start(out=ov, in_=pend[1])
    nc.compile()
    return nc


def _build_gather(repeat: int = 1, bufs: int = 4, variant: str = "ident"):
    nc = bacc.Bacc("TRN2", target_bir_lowering=False, debug=False,
                   num_devices=N_CORES)
    x_ext = nc.dram_tensor("x", [B_LOC, T, C], mybir.dt.float32,
                           kind="ExternalInput").ap()
    idx_ext = nc.dram_tensor("idx", [P, JW], mybir.dt.int16,
                             kind="ExternalInput").ap()
    out_ext = nc.dram_tensor("out", [B_LOC, T, C], mybir.dt.float32,
                             kind="ExternalOutput").ap()

    with tile.TileContext(nc) as tc:
        with tc.tile_pool(name="idxp", bufs=1) as idx_pool, \
             tc.tile_pool(name="data", bufs=bufs) as data_pool:
            nc.gpsimd.load_library(_mlp_lib)
            idx_t = idx_pool.tile([P, JW], mybir.dt.int16)
            nc.sync.dma_start(out=idx_t[:], in_=idx_ext[:])
            for _ in range(repeat):
                for b in range(B_LOC):
                    dt = data_pool.tile([P, J, C], mybir.dt.float32)
                    for g in range(GSPLIT):
                        # gather slot n -> dt[n % 128, n // 128]
                        nc.gpsimd.dma_gather(
                            dt[:, g * J_PER_G:(g + 1) * J_PER_G, :],
                            x_ext[b],
                            idx_t[:, g * JW_PER_G:(g + 1) * JW_PER_G],
                            num_idxs=IDX_PER_G, num_idxs_reg=IDX_PER_G,
                            elem_size=C,
                        )
                    if variant == "block":
                        # host permuted slots so dt[p, j] = out row p*32 + j
                        out_view = out_ext[b].rearrange("(p j) c -> p j c", p=P)
                        nc.sync.dma_start(out=out_view, in_=dt[:])
                    elif variant == "ident":
                        # dt[p, j] = out row j*128 + p: interleaved store
                        out_view = out_ext[b].rearrange("(j p) c -> p j c", p=P)
                        nc.sync.dma_start(out=out_view, in_=dt[:])
                    elif variant == "ident_jstore":
                        ov = out_ext[b].rearrange("(j p) c -> j p c", p=P)
                        for j in range(J):
                            nc.sync.dma_start(out=ov[j], in_=dt[:, j, :])
                    else:
                        raise ValueError(variant)
    nc.compile()
    return nc


def _build(repeat: int = 1, bufs: int | None = None, variant: str | None = None):
    variant = variant or VARIANT
    if variant == "shift":
        return _build_shift(repeat=repeat, bufs=bufs or 2)
    return _build_gather(repeat=repeat, bufs=bufs or 4, variant=variant)


def _prep_idx(indices: np.ndarray, variant: str = "ident") -> np.ndarray:
    idx16 = indices.astype(np.int16)                    # values < 4096 fit
    if variant == "block":
        n = np.arange(T)
        idx16 = idx16[(n % P) * J + (n // P)]
    wrapped = np.ascontiguousarray(idx16.reshape(JW, 16).T)   # [16, JW]
    return np.ascontiguousarray(np.tile(wrapped, (P // 16, 1)))  # [128, JW]


def _prep_masks(indices: np.ndarray):
    d = indices.astype(np.int64) - np.arange(T)
    mm = (d == -1).astype(np.uint8)   # take row t-1
    mp = (d == 1).astype(np.uint8)    # take row t+1
    def expand(m):
        full = np.repeat(m.reshape(P, J), C, axis=1).reshape(P, J, C)
        return np.ascontiguousarray(full)
    return expand(mm), expand(mp)


def _make_in_maps(x: np.ndarray, indices: np.ndarray, variant: str | None = None):
    variant = variant or VARIANT
    x = np.asarray(x)
    indices = np.asarray(indices)
    if variant == "shift":
        mm, mp = _prep_masks(indices)
        maps = []
        for i in range(N_CORES):
            flat = x[i * B_LOC:(i + 1) * B_LOC].reshape(B_LOC * T, C)
            xp = np.zeros((ROWS_PAD, C), dtype=np.float32)
            xp[1:1 + B_LOC * T] = flat
            maps.append({"xp": xp, "mm": mm, "mp": mp})
        return maps
    idx_arr = _prep_idx(indices, variant)
    return [
        {"x": np.ascontiguousarray(x[i * B_LOC:(i + 1) * B_LOC]),
         "idx": idx_arr}
        for i in range(N_CORES)
    ]


def _gather_out(res) -> np.ndarray:
    return np.concatenate([res.results[i]["out"] for i in range(N_CORES)],
                          axis=0)


def _is_jitter(indices: np.ndarray) -> bool:
    idx = np.asarray(indices).astype(np.int64)
    if idx.shape != (T,) or idx.min() < 0 or idx.max() >= T:
        return False
    return bool(np.all(np.abs(idx - np.arange(T)) <= 1))


def kernel(x: np.ndarray, indices: np.ndarray) -> np.ndarray:
    variant = VARIANT if _is_jitter(indices) else "ident"
    if variant not in _CACHE:
        _CACHE[variant] = _build(variant=variant)
    nc = _CACHE[variant]

    in_maps = _make_in_maps(x, indices, variant)
    res = run_bass_kernel_spmd(nc, in_maps, list(range(N_CORES)))
    return _gather_out(res)
